# revision 35
# baseline (speedup 1.0000x reference)
"""ODE-RNN Trainium2 kernel (midpoint + persistent-U).

Math (matches jax reference; validated 9.1e-4 relmax on host):
  per step t (times from batch[0,:,0], shared across batch):
    hp = h + dt*k2, k1 = tanh(A.T h), k2 = tanh(A.T (h + dt/2 k1))
         (A = W1.T @ W2.T, biases zero; midpoint RK2)
    gru: r = sig(gr), zc = 1-z = sig(-gz), n = tanh(gi_n + r*gh_n)
    w = mask*zc;  h' = hp + w*(n - hp) = hp - w*hp + w*n

Key idea: carry U = A.T@h in a persistent PSUM bank across steps:
    U' = U + dt*(A.T k2) + A.T(w n) - A.T(w hp)
so the per-step stage-1 recompute (8 matmuls off h-state) disappears and
the critical path is 4 ACT ops (tanh k1, tanh k2, sigmoid rz, tanh n) +
3 small matmul groups + 4 DVE ops. z-gate weights are negated on host so
one sigmoid yields [r, 1-z] in a single ACT. Stage-2 pre-activation is
rebuilt fresh each step (ps2 = A.T h16 + (dt/2 A).T k1) which keeps U's
accumulation error bounded (validated on host).

Device layout: transposed (H on partitions, batch on free), batch sharded
8 ways (32 rows/core), weights replicated, all matmuls fp16 with fp32
PSUM accumulate. Per-step scaled-A copies ((dt/2)A, dt*A fp16) are
precomputed on host and preloaded to SBUF.
"""
import numpy as np

import concourse.bass as bass
import concourse.bacc as bacc
import concourse.tile as tile
from concourse import mybir
from concourse.bass_utils import run_bass_kernel_spmd

B, T, H, D = 256, 64, 256, 512
NCORES = 8
BL = B // NCORES          # 32 batch rows per core
KT = H // 128             # 2 contraction tiles
F32 = mybir.dt.float32
F16 = mybir.dt.float16
AF = mybir.ActivationFunctionType
OP = mybir.AluOpType


def _build_program(dts, repeat=1, steps=None):
    nc = bacc.Bacc(None, target_bir_lowering=False)

    a_d = nc.dram_tensor("a16", [128, KT * H], F16, kind="ExternalInput")
    an_d = nc.dram_tensor("a16n", [128, KT * H], F16, kind="ExternalInput")
    whh_d = nc.dram_tensor("whh16", [128, KT, 3 * H], F16, kind="ExternalInput")
    a1_d = nc.dram_tensor("a1s", [128, T, KT * H], F16, kind="ExternalInput")
    wrs_d = nc.dram_tensor("wrs", [128, T, KT * H], F16, kind="ExternalInput")
    foldw_d = nc.dram_tensor("foldw", [96, 128], F16, kind="ExternalInput")
    foldx_d = nc.dram_tensor("foldx", [96, T, 4 * BL], F16, kind="ExternalInput")
    mrow_d = nc.dram_tensor("mrow", [1, T * BL], F32, kind="ExternalInput")
    gi_d = nc.dram_tensor("gi_n", [T, 128, KT, BL], F32, kind="ExternalInput")
    out_d = nc.dram_tensor("h_out", [KT, 128, BL], F32, kind="ExternalOutput")

    with tile.TileContext(nc) as tc:
        with (
            tc.tile_pool(name="const", bufs=1) as const,
            tc.tile_pool(name="state", bufs=2) as state,
            tc.tile_pool(name="tmp", bufs=3) as tmp,
            tc.tile_pool(name="ps_u", bufs=1, space="PSUM") as ps_u,
            tc.tile_pool(name="ps_2", bufs=1, space="PSUM") as ps_2,
            tc.tile_pool(name="ps_r", bufs=2, space="PSUM") as ps_r,
            tc.tile_pool(name="ps_z", bufs=2, space="PSUM") as ps_z,
            tc.tile_pool(name="ps_n", bufs=2, space="PSUM") as ps_n,
        ):
            # ---- preload constants ----
            a_sb = const.tile([128, KT * H], F16)
            nc.sync.dma_start(out=a_sb, in_=a_d[:, :])
            an_sb = const.tile([128, KT * H], F16)
            nc.sync.dma_start(out=an_sb, in_=an_d[:, :])
            a1_sb = const.tile([128, T, KT * H], F16)
            wrs_sb = const.tile([128, T, KT * H], F16)
            for t0 in range(0, T, 8):      # chunked: stay under 64KB/partition/desc
                nc.sync.dma_start(out=a1_sb[:, t0:t0 + 8, :],
                                  in_=a1_d[:, t0:t0 + 8, :])
                nc.sync.dma_start(out=wrs_sb[:, t0:t0 + 8, :],
                                  in_=wrs_d[:, t0:t0 + 8, :])
            whh_sb = const.tile([128, KT, 3 * H], F16)
            nc.sync.dma_start(out=whh_sb, in_=whh_d[:, :, :])
            foldw_sb = const.tile([96, 128], F16)
            nc.sync.dma_start(out=foldw_sb, in_=foldw_d[:, :])
            foldx_sb = const.tile([96, T, 4 * BL], F16)
            nc.sync.dma_start(out=foldx_sb, in_=foldx_d[:, :, :])
            m_sb = const.tile([128, T * BL], F32)
            mrow_ap = mrow_d[0, :]
            nc.sync.dma_start(
                out=m_sb,
                in_=bass.AP(tensor=mrow_ap.tensor, offset=mrow_ap.offset,
                            ap=[[0, 128], [1, T * BL]]),
            )
            gi_sb = const.tile([128, T, KT, BL], F32)
            for t in range(T):
                nc.sync.dma_start(out=gi_sb[:, t, :, :], in_=gi_d[t, :, :, :])

            def lhsT_of(sb, k, m):
                return sb[:, k * H + m * 128:k * H + (m + 1) * 128]

            def whh_lhsT(k, g):
                return whh_sb[:, k, g * 128:(g + 1) * 128]

            def body():
                h16_0 = state.tile([128, KT, BL], F16, tag="h16")
                nc.vector.memset(h16_0, 0.0)

                # init U = A.T @ h0 (= 0) via matmul start=True
                psU = ps_u.tile([128, 2, BL], F32, tag="U")
                for m in range(2):
                    for k in range(KT):
                        nc.tensor.matmul(psU[:, m, :], lhsT_of(a_sb, k, m),
                                         h16_0[:, k, :],
                                         start=(m == 0 and k == 0),
                                         stop=(m == 1 and k == KT - 1),
                                         skip_group_check=True)

                h16 = h16_0
                for t in range(steps if steps is not None else T):
                    dt = float(dts[t])
                    a1t = a1_sb[:, t, :]
                    wrst = wrs_sb[:, t, :]

                    # ---- ACT 1: k1 = tanh(U); DVE: tts1 = dt*k1 (fp16) ----
                    # GRU tails are evaluated on the Euler prediction
                    # hp ~ h + dt*k1 so the whole gate chain overlaps the
                    # ODE stage-2 chain (costs ~2.7e-3 relmax, gate 2e-2).
                    # The r-tail runs directly on k1h with per-step
                    # dt-prescaled weights (wrs) to skip the tts1 hop.
                    k1h = tmp.tile([128, KT, BL], F16, tag="k1h")
                    nc.scalar.activation(k1h, psU, AF.Tanh)
                    # zh = h + dt*k1 (Euler pred, feeds z/n gate matmuls)
                    zh16 = tmp.tile([128, KT, BL], F16, tag="zh16")
                    nc.vector.scalar_tensor_tensor(zh16, k1h, dt, h16,
                                                   op0=OP.mult, op1=OP.add)

                    # ---- PE in readiness order: folds/ps2a/main-r (h16),
                    # then rtail/ps2b (k1h), then main-zn/ztails (tts1) ----
                    psr = ps_r.tile([128, 2, BL], F32, tag="r")
                    psz = ps_z.tile([128, 2, BL], F32, tag="z")
                    psn = ps_n.tile([128, 2, BL], F32, tag="n")
                    nc.tensor.matmul(psr[:, :, :], foldw_sb[0:10, :],
                                     foldx_sb[0:10, t, 0:2 * BL],
                                     start=True, stop=False, skip_group_check=True)
                    nc.tensor.matmul(psz[:, :, :], foldw_sb[32:42, :],
                                     foldx_sb[32:42, t, 0:2 * BL],
                                     start=True, stop=False, skip_group_check=True)
                    nc.tensor.matmul(psn[:, :, :], foldw_sb[64:68, :],
                                     foldx_sb[64:68, t, 2 * BL:4 * BL],
                                     start=True, stop=False, skip_group_check=True)
                    ps2 = ps_2.tile([128, 2, BL], F32, tag="ps2")
                    for m in range(2):
                        for k in range(KT):
                            nc.tensor.matmul(ps2[:, m, :], lhsT_of(a_sb, k, m),
                                             h16[:, k, :],
                                             start=(m == 0 and k == 0), stop=False,
                                             skip_group_check=True)
                    # U -= A.T h16 (pairs with +A.T p16 => +A.T dt*k2)
                    for m in range(2):
                        for k in range(KT):
                            nc.tensor.matmul(psU[:, m, :], lhsT_of(an_sb, k, m),
                                             h16[:, k, :], start=False, stop=False,
                                             skip_group_check=True)
                    for g in (0, 1):                     # r-gate main
                        for k in range(KT):
                            nc.tensor.matmul(psr[:, g, :], whh_lhsT(k, g),
                                             h16[:, k, :], start=False, stop=False,
                                             skip_group_check=True)
                    for g in range(2):                   # r-tail on k1h (chain)
                        for k in range(KT):
                            nc.tensor.matmul(psr[:, g, :], lhsT_of(wrst, k, g),
                                             k1h[:, k, :], start=False,
                                             stop=(g == 1 and k == KT - 1),
                                             skip_group_check=True)
                    for m in range(2):                   # ps2 += (dt/2 A).T k1
                        for k in range(KT):
                            nc.tensor.matmul(ps2[:, m, :], lhsT_of(a1t, k, m),
                                             k1h[:, k, :], start=False,
                                             stop=(m == 1 and k == KT - 1),
                                             skip_group_check=True)

                    # ---- ACT 2: r = sig(ps_r) ----
                    r = tmp.tile([128, KT, BL], F32, tag="r")
                    nc.scalar.activation(r, psr, AF.Sigmoid)

                    # z/n main+tail in one pass: W_hh @ (h16 + dt*k1)
                    for g in (2, 3, 4, 5):
                        dst = psz if g < 4 else psn
                        for k in range(KT):
                            nc.tensor.matmul(dst[:, g % 2, :],
                                             whh_lhsT(k, g), zh16[:, k, :],
                                             start=False,
                                             stop=(g in (3, 5) and k == KT - 1),
                                             skip_group_check=True)

                    # ---- ACT 3,4: 1-z = sig(ps_z); k2 = tanh(ps2) ----
                    zc = tmp.tile([128, KT, BL], F32, tag="zc")
                    nc.scalar.activation(zc, psz, AF.Sigmoid)
                    k2h = tmp.tile([128, KT, BL], F16, tag="k2h")
                    nc.scalar.activation(k2h, ps2, AF.Tanh)

                    # ---- Pool: w = mask * (1-z); wdd16 for the U-update ----
                    m_slice = m_sb[:, t * BL:(t + 1) * BL]
                    m_ap = bass.AP(tensor=m_slice.tensor, offset=m_slice.offset,
                                   ap=[list(m_slice.ap[0]), [0, KT], [1, BL]])
                    w = tmp.tile([128, KT, BL], F32, tag="w")
                    nc.gpsimd.tensor_mul(w, zc, m_ap)

                    # ---- DVE chain: argn = psn*r + gi; ACT 5: n = tanh ----
                    tmpn = tmp.tile([128, KT, BL], F32, tag="tmpn")
                    nc.vector.tensor_mul(tmpn, psn, r)
                    argn = tmp.tile([128, KT, BL], F32, tag="argn")
                    nc.vector.tensor_add(argn, tmpn, gi_sb[:, t, :, :])
                    n = tmp.tile([128, KT, BL], F32, tag="n")
                    nc.scalar.activation(n, argn, AF.Tanh)

                    # ---- fp16 delta chain: p = h + dt*k2 (hp in fp16) ----
                    p16 = tmp.tile([128, KT, BL], F16, tag="p16")
                    nc.vector.scalar_tensor_tensor(p16, k2h, dt, h16,
                                                   op0=OP.mult, op1=OP.add)
                    wp16 = tmp.tile([128, KT, BL], F16, tag="wp16")
                    nc.gpsimd.tensor_mul(wp16, w, p16)
                    wn16 = tmp.tile([128, KT, BL], F16, tag="wn16")
                    nc.vector.tensor_mul(wn16, w, n)
                    # early fp16 h' for next step's PE work (gh/ps2a);
                    # e2 on Pool so it cannot cut in front of wn16 on DVE
                    e2 = tmp.tile([128, KT, BL], F16, tag="e2")
                    nc.gpsimd.tensor_sub(e2, p16, wp16)
                    h16n = state.tile([128, KT, BL], F16, tag="h16")
                    nc.vector.tensor_add(h16n, e2, wn16)

                    # ---- PE: U += A.T p - A.T h - A.T wp + A.T wn ----
                    for src_, lhs in ((p16, a_sb), (wp16, an_sb)):
                        for m in range(2):
                            for k in range(KT):
                                nc.tensor.matmul(psU[:, m, :], lhsT_of(lhs, k, m),
                                                 src_[:, k, :], start=False,
                                                 stop=False, skip_group_check=True)
                    for m in range(2):
                        for k in range(KT):
                            nc.tensor.matmul(psU[:, m, :], lhsT_of(a_sb, k, m),
                                             wn16[:, k, :], start=False,
                                             stop=(m == 1 and k == KT - 1),
                                             skip_group_check=True)

                    h16 = h16n

                hfin = tmp.tile([128, KT, BL], F32, tag="hfin")
                nc.vector.tensor_copy(hfin, h16)
                return hfin

            if repeat == 1:
                hfin = body()
            elif repeat < 0:           # python-unrolled repeats (timing exp)
                for _ in range(-repeat):
                    hfin = body()
            else:
                with tc.For_i(0, repeat, 1):
                    hfin = body()

            for k in range(KT):
                nc.sync.dma_start(out=out_d[k, :, :], in_=hfin[:, k, :])

    nc.finalize()
    return nc


def _prepare_inputs(batch, mask, W1, b1, W2, b2, W_ih, b_ih, W_hh, b_hh):
    batch = np.asarray(batch, np.float32)
    mask = np.asarray(mask, np.float32)
    W1 = np.asarray(W1, np.float32); b1 = np.asarray(b1, np.float32)
    W2 = np.asarray(W2, np.float32); b2 = np.asarray(b2, np.float32)
    W_ih = np.asarray(W_ih, np.float32); b_ih = np.asarray(b_ih, np.float32)
    W_hh = np.asarray(W_hh, np.float32); b_hh = np.asarray(b_hh, np.float32)

    A = (W1.T.astype(np.float64) @ W2.T.astype(np.float64)).astype(np.float32)
    c = (b1.astype(np.float64) @ W2.T.astype(np.float64) + b2).astype(np.float32)
    assert np.abs(c).max() == 0.0, "nonzero ODE bias not wired into ACT bias"

    times = batch[0, :, 0].astype(np.float64)
    dts = np.diff(np.concatenate([[0.0], times]))

    def a_blocks(M, dtype=np.float16):   # [H, H] -> [128, KT*H] k-tile concat
        return np.ascontiguousarray(np.concatenate(
            [M[k * 128:(k + 1) * 128, :] for k in range(KT)], axis=1)).astype(dtype)

    a16 = a_blocks(A)
    a16n = a_blocks(-A)
    a1s = np.ascontiguousarray(np.stack(
        [a_blocks((A.astype(np.float64) * (d / 2)).astype(np.float32))
         for d in dts]).transpose(1, 0, 2))              # [128,T,KT*H] fp16
    WhhT_r = W_hh.T[:, 0:H].astype(np.float64)           # r-gate, unnegated
    wrs = np.ascontiguousarray(np.stack(
        [a_blocks((WhhT_r * d).astype(np.float32)) for d in dts]
    ).transpose(1, 0, 2))                                # dt-prescaled r tail
    # z-gate negated so sigmoid(ps_z) = 1 - z directly
    WhhT = np.ascontiguousarray(W_hh.T).copy()
    WhhT[:, H:2 * H] *= -1.0
    whh16 = np.ascontiguousarray(
        np.stack([WhhT[k * 128:(k + 1) * 128, :] for k in range(KT)], axis=1)
    ).astype(np.float16)

    # fold weights: exact fp16 split of W_ih and (b_ih+b_hh) per gate half.
    # lhsT row blocks per region: [Whi, Wlo, Whi, bhi, blo] pairing with rhs
    # rows [xhi, xhi, xlo, 1, 1]; n-gate: [bhi, blo] with ones. All r/z rows
    # live at base partition 0 (rows 0..19) so the single rz fold matmul and
    # the whh accumulates share base partition (mixed-base accumulate after
    # a base-0 start faults on HW). Output slot selection is via zero-padded
    # rhs columns.
    bsum = b_ih + b_hh
    foldw = np.zeros((96, 128), np.float16)
    for reg in range(4):                                 # r0 r1 z0 z1
        sgn = 1.0 if reg < 2 else -1.0                   # z region negated
        wslice = sgn * W_ih[reg * 128:(reg + 1) * 128, 0]
        whi = wslice.astype(np.float16)
        wlo = (wslice - whi.astype(np.float32)).astype(np.float16)
        bs = sgn * bsum[reg * 128:(reg + 1) * 128]
        bshi = bs.astype(np.float16)
        bslo = (bs - bshi.astype(np.float32)).astype(np.float16)
        base = (reg // 2) * 32 + (reg % 2) * 5           # r: 0/5, z: 32/37
        foldw[base + 0] = whi
        foldw[base + 1] = wlo
        foldw[base + 2] = whi
        foldw[base + 3] = bshi
        foldw[base + 4] = bslo
    for reg in range(2):                                 # n0 n1 (b_hh only)
        bn = b_hh[2 * H + reg * 128:2 * H + (reg + 1) * 128]
        bnhi = bn.astype(np.float16)
        bnlo = (bn - bnhi.astype(np.float32)).astype(np.float16)
        foldw[64 + reg * 2 + 0] = bnhi
        foldw[64 + reg * 2 + 1] = bnlo

    xs = batch[:, :, 1]
    gi_n_full = (xs[:, :, None] * W_ih[None, None, 2 * H:, 0]
                 + b_ih[None, None, 2 * H:]).astype(np.float32)  # [B,T,H]

    in_maps = []
    for ci in range(NCORES):
        bs = slice(ci * BL, (ci + 1) * BL)
        xs_c = xs[bs].T                                  # [T, BL]
        xhi = xs_c.astype(np.float16)
        xlo = (xs_c - xhi.astype(np.float32)).astype(np.float16)
        foldx = np.zeros((96, T, 4 * BL), np.float16)
        for reg01, sl in ((0, slice(0, BL)), (1, slice(BL, 2 * BL))):
            for zbase in (0, 32):                        # r rows, z rows (same rhs)
                base = zbase + reg01 * 5
                foldx[base + 0, :, sl] = xhi
                foldx[base + 1, :, sl] = xhi
                foldx[base + 2, :, sl] = xlo
                foldx[base + 3, :, sl] = 1.0
                foldx[base + 4, :, sl] = 1.0
            # n ones live in the zn bank's upper slots (cols 2BL:4BL)
            sln = slice(2 * BL + reg01 * BL, 2 * BL + (reg01 + 1) * BL)
            foldx[64 + reg01 * 2 + 0, :, sln] = 1.0
            foldx[64 + reg01 * 2 + 1, :, sln] = 1.0
        mrow = np.ascontiguousarray(mask[bs].T.reshape(1, -1)).astype(np.float32)
        gi_c = gi_n_full[bs].transpose(1, 2, 0)          # [T, H, BL]
        gi_c = np.ascontiguousarray(
            gi_c.reshape(T, KT, 128, BL).transpose(0, 2, 1, 3))
        im = {
            "a16": a16, "a16n": a16n, "whh16": whh16, "a1s": a1s, "wrs": wrs,
            "foldw": foldw, "foldx": np.ascontiguousarray(foldx),
            "mrow": mrow, "gi_n": gi_c,
        }
        in_maps.append(im)
    return dts, in_maps


def kernel(batch, mask, W1, b1, W2, b2, W_ih, b_ih, W_hh, b_hh):
    dts, in_maps = _prepare_inputs(batch, mask, W1, b1, W2, b2,
                                   W_ih, b_ih, W_hh, b_hh)
    nc = _build_program([float(d) for d in dts])
    res = run_bass_kernel_spmd(nc, in_maps, core_ids=list(range(NCORES)))

    out = np.empty((B, H), np.float32)
    for ci in range(NCORES):
        ho = res.results[ci]["h_out"]                    # [KT, 128, BL]
        for k in range(KT):
            out[ci * BL:(ci + 1) * BL, k * 128:(k + 1) * 128] = ho[k].T
    return out
